# revision 4
# baseline (speedup 1.0000x reference)
"""Trainium2 Bass kernel for nn_DistributionLossWithLabel.

Reference computation (B=8192, C=64):
    lq = log(q); lp = log(p)
    positive[i] = mean_c p[i,c]*(lp[i,c]-lq[i,c])
    a[j]        = sum_c p[j,c]*lp[j,c] / C
    kl[i,j]     = a[j] - (lq @ p^T)[i,j] / C
    negative[i] = sum_j kl[i,j] + sum_j kl[i,j]*(1-L[i,j])
                = 2*sum_j kl[i,j] - sum_j kl[i,j]*L[i,j]
    loss        = sum_i positive[i]/negative[i]

Key reformulation used on device (per core, rows i sharded 8 ways):
    sum_j kl[i,j] = S_a - (lq[i,:]/C) . P      (closed form; no tiles needed)
        with P[c] = sum_j p[j,c],  S_a = sum_j a[j]
    kl tile [128i x 1024j] = one K=128 matmul:
        lhsT = [ -lq^T/C ; (1/C)*ones ]  (128 x i)
        rhs  = [ p^T     ; (p*lp)^T   ]  (128 x j)
    rowdot[i] = sum_j kl[i,j]*L[i,j] via ONE fused DVE op per tile
        (tensor_tensor_reduce: out=kl*L, accum_out=row sums)
    negative = 2*rowsumkl - rowdot ; loss partial = sum positive/negative

The only large input is labels (32MB/core) -> kernel is DMA-bound, every
other engine stays under the DMA roofline.
"""

import sys

if "/opt/trn_rl_repo" not in sys.path:
    sys.path.insert(0, "/opt/trn_rl_repo")

import numpy as np

import concourse.bass as bass
import concourse.tile as tile
from concourse import bacc, mybir
from concourse.masks import make_identity

FP = mybir.dt.float32
AF = mybir.ActivationFunctionType
ALU = mybir.AluOpType
AX = mybir.AxisListType

B_FULL = 8192
C = 64
N_CORES = 8


def build_nc(B=B_FULL, shard=B_FULL // N_CORES, debug=False):
    """Build the single-core SPMD Bass program.

    B: total rows (columns j of the labels shard), multiple of 1024.
    shard: rows per core (i), multiple of 128.
    """
    assert B % 1024 == 0 and shard % 128 == 0
    nblk = shard // 128        # i-blocks of 128 rows
    njc = B // 128             # 128-row chunks of p (for transposes)
    njt = B // 1024            # j-tiles of 1024 for the main loop
    nld = B // 4096 if B % 4096 == 0 else 0  # L DMA tiles of 4096 cols
    if nld == 0:
        nld, ldw = B // 1024, 1024
    else:
        ldw = 4096
    kl_per_ld = ldw // 1024

    nc = bacc.Bacc("TRN2", target_bir_lowering=False, debug=debug)

    q_d = nc.dram_tensor("q", [shard, C], FP, kind="ExternalInput")
    p_d = nc.dram_tensor("p", [B, C], FP, kind="ExternalInput")
    pmy_d = nc.dram_tensor("p_my", [shard, C], FP, kind="ExternalInput")
    lab_d = nc.dram_tensor("labels", [shard, B], FP, kind="ExternalInput")
    out_d = nc.dram_tensor("out", [128, 1], FP, kind="ExternalOutput")

    rcpC = 1.0 / C

    with tile.TileContext(nc) as tc:
        with (
            tc.tile_pool(name="const", bufs=1) as cp,
            tc.tile_pool(name="lpool", bufs=4) as lp_pool,
            tc.tile_pool(name="spool", bufs=2) as sp,
            tc.tile_pool(name="tp_ps", bufs=2, space="PSUM") as tp_ps,
            tc.tile_pool(name="kl_ps", bufs=2, space="PSUM") as kl_ps,
            tc.tile_pool(name="sm_ps", bufs=2, space="PSUM") as sm_ps,
        ):
            # ---------------- constants ----------------
            ident = cp.tile([128, 128], FP)
            make_identity(nc, ident[:])
            identS = cp.tile([128, 128], FP)
            # scaled identity (-1/C) used to transpose+scale the q blocks
            nc.scalar.mul(identS[:], ident[:], -rcpC)
            ones_col = cp.tile([128, 1], FP)
            nc.vector.memset(ones_col[:], 1.0)
            ones_row = cp.tile([1, 64], FP)
            nc.vector.memset(ones_row[:], 1.0)

            # ---------------- p side: rhs = [p^T ; (p*lp)^T] ----------------
            # PA holds, per 128-row chunk ch: cols [0:64) = p rows, cols
            # [64:128) = p*log(p) rows.  Layout matches a [128,128] PE
            # transpose producing the rhs block directly.
            PA = cp.tile([128, njc * 128], FP)
            PAr = PA[:].rearrange("p (n k) -> p n k", k=128)
            p_slots = PAr[:, :, 0:64]
            a_slots = PAr[:, :, 64:128]
            nc.sync.dma_start(
                out=p_slots, in_=p_d.ap().rearrange("(n p) c -> p n c", p=128)
            )
            LPf = cp.tile([128, njc * 64], FP)
            LPr = LPf[:].rearrange("p (n c) -> p n c", c=64)
            nc.scalar.activation(LPr, p_slots, AF.Ln)
            nc.vector.tensor_tensor(a_slots, p_slots, LPr, op=ALU.mult)

            rhs = cp.tile([128, B], FP)
            for grp in range(njc // 4):
                tp = tp_ps.tile([128, 512], FP, tag="tp")
                for k in range(4):
                    ch = grp * 4 + k
                    nc.tensor.transpose(
                        tp[:, k * 128 : (k + 1) * 128],
                        PA[:, ch * 128 : (ch + 1) * 128],
                        ident[:],
                    )
                dst = rhs[:, grp * 512 : (grp + 1) * 512]
                if grp % 2 == 0:
                    nc.scalar.copy(dst, tp[:])
                else:
                    nc.vector.tensor_copy(dst, tp[:])

            # ---------------- q side: lhsT = [-lq^T/C ; (1/C)] ----------------
            QRAW = cp.tile([128, nblk * 64], FP)
            nc.sync.dma_start(
                out=QRAW[:].rearrange("p (n c) -> p n c", c=64),
                in_=q_d.ap().rearrange("(n p) c -> p n c", p=128),
            )
            QN = cp.tile([128, nblk * 128], FP)
            nc.vector.memset(QN[:], -1.0)
            QNr = QN[:].rearrange("p (n k) -> p n k", k=128)
            lq_slots = QNr[:, :, 0:64]
            nc.scalar.activation(
                lq_slots, QRAW[:].rearrange("p (n c) -> p n c", c=64), AF.Ln
            )
            lhsT = cp.tile([128, nblk * 128], FP)
            for grp in range((nblk + 3) // 4):
                k0 = grp * 4
                kn = min(4, nblk - k0)
                tq = tp_ps.tile([128, 512], FP, tag="tp")
                for k in range(kn):
                    ch = k0 + k
                    nc.tensor.matmul(
                        tq[:, k * 128 : (k + 1) * 128],
                        QN[:, ch * 128 : (ch + 1) * 128],
                        identS[:],
                        start=True,
                        stop=True,
                    )
                nc.scalar.copy(
                    lhsT[:, k0 * 128 : (k0 + kn) * 128], tq[:, 0 : kn * 128]
                )

            # ---------------- rsmall = [P[c] ; S_a] ----------------
            cs_ps = sm_ps.tile([128, 1], FP, tag="sm")
            for ch in range(njc):
                nc.tensor.matmul(
                    cs_ps[:],
                    PA[:, ch * 128 : (ch + 1) * 128],
                    ones_col[:],
                    start=(ch == 0),
                    stop=(ch == njc - 1),
                )
            rsmall = cp.tile([128, 1], FP)
            nc.scalar.copy(rsmall[:], cs_ps[:])
            # T[c] = sum_j (p*lp)[j,c] sits in rows 64:128; S_a = sum_c T[c]/C
            s_ps = sm_ps.tile([1, 1], FP, tag="sm")
            nc.tensor.matmul(
                s_ps[:], rsmall[64:128, :], ones_col[64:128, :], start=True, stop=True
            )
            s_sb = cp.tile([1, 1], FP)
            nc.scalar.copy(s_sb[:], s_ps[:])
            b_ps = sm_ps.tile([64, 1], FP, tag="sm")
            nc.tensor.matmul(b_ps[:], ones_row[:], s_sb[:], start=True, stop=True)
            nc.scalar.activation(rsmall[64:128, :], b_ps[:], AF.Copy, scale=rcpC)

            # ---------------- per-block rowsumkl (closed form) ----------------
            rskl_sb = cp.tile([128, nblk], FP)
            for blk in range(nblk):
                rs_ps = sm_ps.tile([128, 1], FP, tag="sm")
                nc.tensor.matmul(
                    rs_ps[:],
                    lhsT[:, blk * 128 : (blk + 1) * 128],
                    rsmall[:],
                    start=True,
                    stop=True,
                )
                nc.scalar.copy(rskl_sb[:, blk : blk + 1], rs_ps[:])

            # ---------------- positive ----------------
            Pmy = cp.tile([128, nblk * 64], FP)
            nc.sync.dma_start(
                out=Pmy[:].rearrange("p (n c) -> p n c", c=64),
                in_=pmy_d.ap().rearrange("(n p) c -> p n c", p=128),
            )
            LPmy = cp.tile([128, nblk * 64], FP)
            nc.scalar.activation(LPmy[:], Pmy[:], AF.Ln)
            tsub = cp.tile([128, nblk * 64], FP)
            nc.vector.tensor_tensor(
                tsub[:].rearrange("p (n c) -> p n c", c=64),
                LPmy[:].rearrange("p (n c) -> p n c", c=64),
                lq_slots,
                op=ALU.subtract,
            )
            pos_sb = cp.tile([128, nblk], FP)
            for blk in range(nblk):
                pscr = sp.tile([128, 64], FP, tag="pscr")
                nc.vector.scalar_tensor_tensor(
                    out=pscr[:],
                    in0=Pmy[:, blk * 64 : (blk + 1) * 64],
                    scalar=rcpC,
                    in1=tsub[:, blk * 64 : (blk + 1) * 64],
                    op0=ALU.mult,
                    op1=ALU.mult,
                    accum_out=pos_sb[:, blk : blk + 1],
                )

            # ---------------- main loop ----------------
            rd_cols = cp.tile([128, nblk * njt], FP)
            lab_ap = lab_d.ap()
            for blk in range(nblk):
                lhsT_blk = lhsT[:, blk * 128 : (blk + 1) * 128]
                for d in range(nld):
                    L = lp_pool.tile([128, ldw], FP, tag="L")
                    nc.sync.dma_start(
                        out=L[:],
                        in_=lab_ap[
                            blk * 128 : (blk + 1) * 128, d * ldw : (d + 1) * ldw
                        ],
                    )
                    for h in range(kl_per_ld):
                        jt = d * kl_per_ld + h
                        kl = kl_ps.tile([128, 1024], FP, tag="kl")
                        for half in range(2):
                            j0 = jt * 1024 + half * 512
                            nc.tensor.matmul(
                                kl[:, half * 512 : (half + 1) * 512],
                                lhsT_blk,
                                rhs[:, j0 : j0 + 512],
                                start=True,
                                stop=True,
                            )
                        scr = sp.tile([128, 1024], FP, tag="scr")
                        nc.vector.scalar_tensor_tensor(
                            out=scr[:],
                            in0=kl[:],
                            scalar=1.0,
                            in1=L[:, h * 1024 : (h + 1) * 1024],
                            op0=ALU.mult,
                            op1=ALU.mult,
                            accum_out=rd_cols[:, blk * njt + jt : blk * njt + jt + 1],
                        )

            # ---------------- epilogue ----------------
            rd8 = cp.tile([128, nblk], FP)
            nc.vector.reduce_sum(
                rd8[:],
                rd_cols[:].rearrange("p (b j) -> p b j", j=njt),
                axis=AX.X,
            )
            neg8 = cp.tile([128, nblk], FP)
            nc.vector.scalar_tensor_tensor(
                out=neg8[:],
                in0=rskl_sb[:],
                scalar=2.0,
                in1=rd8[:],
                op0=ALU.mult,
                op1=ALU.subtract,
            )
            rec8 = cp.tile([128, nblk], FP)
            nc.vector.reciprocal(rec8[:], neg8[:])
            r8 = cp.tile([128, nblk], FP)
            nc.vector.tensor_tensor(r8[:], pos_sb[:], rec8[:], op=ALU.mult)
            out_col = cp.tile([128, 1], FP)
            nc.vector.reduce_sum(out_col[:], r8[:], axis=AX.X)
            nc.sync.dma_start(out=out_d.ap(), in_=out_col[:])

    nc.compile()
    return nc


_NC_CACHE = {}


def _get_nc(B, shard):
    key = (B, shard)
    if key not in _NC_CACHE:
        _NC_CACHE[key] = build_nc(B, shard)
    return _NC_CACHE[key]


def make_in_maps(q, p, labels_matrix, n_cores=N_CORES):
    B = q.shape[0]
    shard = B // n_cores
    maps = []
    for k in range(n_cores):
        s = slice(k * shard, (k + 1) * shard)
        maps.append(
            {
                "q": np.ascontiguousarray(q[s]),
                "p": np.ascontiguousarray(p),
                "p_my": np.ascontiguousarray(p[s]),
                "labels": np.ascontiguousarray(labels_matrix[s]),
            }
        )
    return maps


def kernel(q, p, labels_matrix):
    from concourse.bass_utils import run_bass_kernel_spmd

    q = np.asarray(q, dtype=np.float32)
    p = np.asarray(p, dtype=np.float32)
    labels_matrix = np.asarray(labels_matrix, dtype=np.float32)
    B = q.shape[0]
    shard = B // N_CORES
    nc = _get_nc(B, shard)
    in_maps = make_in_maps(q, p, labels_matrix, N_CORES)
    res = run_bass_kernel_spmd(nc, in_maps, core_ids=list(range(N_CORES)))
    total = 0.0
    for r in res.results:
        total += r["out"].astype(np.float64).sum()
    return np.float32(total)


# revision 9
# speedup vs baseline: 1.2457x; 1.2457x over previous
"""Trainium2 Bass kernel for nn_DistributionLossWithLabel.

Reference computation (B=8192, C=64):
    lq = log(q); lp = log(p)
    positive[i] = mean_c p[i,c]*(lp[i,c]-lq[i,c])
    a[j]        = sum_c p[j,c]*lp[j,c] / C
    kl[i,j]     = a[j] - (lq @ p^T)[i,j] / C
    negative[i] = sum_j kl[i,j] + sum_j kl[i,j]*(1-L[i,j])
                = 2*sum_j kl[i,j] - sum_j kl[i,j]*L[i,j]
    loss        = sum_i positive[i]/negative[i]

Key reformulation used on device (per core, rows i sharded 8 ways):
    sum_j kl[i,j] = S_a - (lq[i,:]/C) . P      (closed form; no tiles needed)
        with P[c] = sum_j p[j,c],  S_a = sum_j a[j]
    kl tile [128i x 1024j] = one K=128 matmul:
        lhsT = [ -lq^T/C ; (1/C)*ones ]  (128 x i)
        rhs  = [ p^T     ; (p*lp)^T   ]  (128 x j)
    rowdot[i] = sum_j kl[i,j]*L[i,j] via ONE fused DVE op per tile
        (tensor_tensor_reduce: out=kl*L, accum_out=row sums)
    negative = 2*rowsumkl - rowdot ; loss partial = sum positive/negative

The only large input is labels (32MB/core) -> kernel is DMA-bound, every
other engine stays under the DMA roofline.
"""

import sys

if "/opt/trn_rl_repo" not in sys.path:
    sys.path.insert(0, "/opt/trn_rl_repo")

import numpy as np

import concourse.bass as bass
import concourse.tile as tile
from concourse import bacc, mybir
from concourse.masks import make_identity

FP = mybir.dt.float32
BF = mybir.dt.bfloat16
AF = mybir.ActivationFunctionType
ALU = mybir.AluOpType
AX = mybir.AxisListType

B_FULL = 8192
C = 64
N_CORES = 8


def build_nc(B=B_FULL, shard=B_FULL // N_CORES, debug=False):
    """Build the single-core SPMD Bass program.

    B: total rows (columns j of the labels shard), multiple of 1024.
    shard: rows per core (i), multiple of 128.
    """
    assert B % 1024 == 0 and shard % 128 == 0
    nblk = shard // 128        # i-blocks of 128 rows
    njc = B // 128             # 128-row chunks of p (for transposes)
    njt = B // 1024            # j-tiles of 1024 for the main loop
    nld = B // 4096 if B % 4096 == 0 else 0  # L DMA tiles of 4096 cols
    if nld == 0:
        nld, ldw = B // 1024, 1024
    else:
        ldw = 4096
    kl_per_ld = ldw // 1024

    nc = bacc.Bacc("TRN2", target_bir_lowering=False, debug=debug)

    q_d = nc.dram_tensor("q", [shard, C], FP, kind="ExternalInput")
    p_d = nc.dram_tensor("p", [B, C], FP, kind="ExternalInput")
    pmy_d = nc.dram_tensor("p_my", [shard, C], FP, kind="ExternalInput")
    lab_d = nc.dram_tensor("labels", [shard, B], FP, kind="ExternalInput")
    out_d = nc.dram_tensor("out", [128, 1], FP, kind="ExternalOutput")

    rcpC = 1.0 / C

    with tile.TileContext(nc) as tc:
        with (
            tc.tile_pool(name="const", bufs=1) as cp,
            tc.tile_pool(name="lpool", bufs=4) as lp_pool,
            tc.tile_pool(name="spool", bufs=2) as sp,
            tc.tile_pool(name="tp_ps", bufs=2, space="PSUM") as tp_ps,
            tc.tile_pool(name="kl_ps", bufs=2, space="PSUM") as kl_ps,
            tc.tile_pool(name="sm_ps", bufs=2, space="PSUM") as sm_ps,
        ):
            # ---------------- constants ----------------
            ident = cp.tile([128, 128], FP)
            make_identity(nc, ident[:])
            identS = cp.tile([128, 128], FP)
            # scaled identity (-1/C) used to transpose+scale the q blocks
            nc.scalar.mul(identS[:], ident[:], -rcpC)
            ones_col = cp.tile([128, 1], FP)
            nc.vector.memset(ones_col[:], 1.0)
            ones_row = cp.tile([1, 64], FP)
            nc.vector.memset(ones_row[:], 1.0)

            # ---------------- p side: rhs = [p^T ; (p*lp)^T] ----------------
            # PA holds, per 128-row chunk ch: cols [0:64) = p rows, cols
            # [64:128) = p*log(p) rows.  Layout matches a [128,128] PE
            # transpose producing the rhs block directly.
            PA = cp.tile([128, njc * 128], FP)
            PAr = PA[:].rearrange("p (n k) -> p n k", k=128)
            p_slots = PAr[:, :, 0:64]
            a_slots = PAr[:, :, 64:128]
            nc.sync.dma_start(
                out=p_slots, in_=p_d.ap().rearrange("(n p) c -> p n c", p=128)
            )
            LPf = cp.tile([128, njc * 64], FP)
            LPr = LPf[:].rearrange("p (n c) -> p n c", c=64)
            nc.scalar.activation(LPr, p_slots, AF.Ln)
            nc.vector.tensor_tensor(a_slots, p_slots, LPr, op=ALU.mult)

            # rhs in bf16 for the main matmuls (fp32 matmul on trn2 lowers
            # to 2 HW passes at ~2.8x the per-pass cost; bf16 keeps the
            # final loss within ~2e-5 because negative's dominant term
            # (rowsumkl) stays on an exact fp32 closed-form path)
            rhs = cp.tile([128, B], BF)
            for grp in range(njc // 4):
                tp = tp_ps.tile([128, 512], FP, tag="tp")
                for k in range(4):
                    ch = grp * 4 + k
                    nc.tensor.transpose(
                        tp[:, k * 128 : (k + 1) * 128],
                        PA[:, ch * 128 : (ch + 1) * 128],
                        ident[:],
                    )
                dst = rhs[:, grp * 512 : (grp + 1) * 512]
                if grp % 2 == 0:
                    nc.scalar.copy(dst, tp[:])
                else:
                    nc.vector.tensor_copy(dst, tp[:])

            # ---------------- q side: lhsT = [-lq^T/C ; (1/C)] ----------------
            QRAW = cp.tile([128, nblk * 64], FP)
            nc.sync.dma_start(
                out=QRAW[:].rearrange("p (n c) -> p n c", c=64),
                in_=q_d.ap().rearrange("(n p) c -> p n c", p=128),
            )
            QN = cp.tile([128, nblk * 128], FP)
            nc.vector.memset(QN[:], -1.0)
            QNr = QN[:].rearrange("p (n k) -> p n k", k=128)
            lq_slots = QNr[:, :, 0:64]
            nc.scalar.activation(
                lq_slots, QRAW[:].rearrange("p (n c) -> p n c", c=64), AF.Ln
            )
            # two copies of lhsT: fp32 for the exact rowsumkl matvec, bf16
            # for the main-loop matmuls
            lhsT = cp.tile([128, nblk * 128], FP)
            lhsT16 = cp.tile([128, nblk * 128], BF)
            for grp in range((nblk + 3) // 4):
                k0 = grp * 4
                kn = min(4, nblk - k0)
                tq = tp_ps.tile([128, 512], FP, tag="tp")
                for k in range(kn):
                    ch = k0 + k
                    nc.tensor.matmul(
                        tq[:, k * 128 : (k + 1) * 128],
                        QN[:, ch * 128 : (ch + 1) * 128],
                        identS[:],
                        start=True,
                        stop=True,
                    )
                nc.scalar.copy(
                    lhsT[:, k0 * 128 : (k0 + kn) * 128], tq[:, 0 : kn * 128]
                )
                nc.vector.tensor_copy(
                    lhsT16[:, k0 * 128 : (k0 + kn) * 128], tq[:, 0 : kn * 128]
                )

            # ---------------- rsmall = [P[c] ; S_a] ----------------
            # fold the 64 chunks with one strided DVE reduce (PE N=1
            # accumulate matmuls here measured ~73us of PE queue time),
            # then a single partition-reduce matmul
            colsum128 = cp.tile([128, 128], FP)
            nc.vector.reduce_sum(
                colsum128[:],
                PA[:].rearrange("p (n k) -> p k n", k=128),
                axis=AX.X,
            )
            cs_ps = sm_ps.tile([128, 1], FP, tag="sm")
            nc.tensor.matmul(
                cs_ps[:], colsum128[:], ones_col[:], start=True, stop=True
            )
            rsmall = cp.tile([128, 1], FP)
            nc.scalar.copy(rsmall[:], cs_ps[:])
            # T[c] = sum_j (p*lp)[j,c] sits in rows 64:128; S_a = sum_c T[c]/C
            s_ps = sm_ps.tile([1, 1], FP, tag="sm")
            nc.tensor.matmul(
                s_ps[:], rsmall[64:128, :], ones_col[64:128, :], start=True, stop=True
            )
            s_sb = cp.tile([1, 1], FP)
            nc.scalar.copy(s_sb[:], s_ps[:])
            b_ps = sm_ps.tile([64, 1], FP, tag="sm")
            nc.tensor.matmul(b_ps[:], ones_row[:], s_sb[:], start=True, stop=True)
            nc.scalar.activation(rsmall[64:128, :], b_ps[:], AF.Copy, scale=rcpC)

            # ---------------- per-block rowsumkl (closed form) ----------------
            rskl_sb = cp.tile([128, nblk], FP)
            for blk in range(nblk):
                rs_ps = sm_ps.tile([128, 1], FP, tag="sm")
                nc.tensor.matmul(
                    rs_ps[:],
                    lhsT[:, blk * 128 : (blk + 1) * 128],
                    rsmall[:],
                    start=True,
                    stop=True,
                )
                nc.scalar.copy(rskl_sb[:, blk : blk + 1], rs_ps[:])

            # ---------------- positive ----------------
            Pmy = cp.tile([128, nblk * 64], FP)
            nc.sync.dma_start(
                out=Pmy[:].rearrange("p (n c) -> p n c", c=64),
                in_=pmy_d.ap().rearrange("(n p) c -> p n c", p=128),
            )
            LPmy = cp.tile([128, nblk * 64], FP)
            nc.scalar.activation(LPmy[:], Pmy[:], AF.Ln)
            tsub = cp.tile([128, nblk * 64], FP)
            nc.vector.tensor_tensor(
                tsub[:].rearrange("p (n c) -> p n c", c=64),
                LPmy[:].rearrange("p (n c) -> p n c", c=64),
                lq_slots,
                op=ALU.subtract,
            )
            pos_sb = cp.tile([128, nblk], FP)
            for blk in range(nblk):
                pscr = sp.tile([128, 64], FP, tag="pscr")
                nc.vector.scalar_tensor_tensor(
                    out=pscr[:],
                    in0=Pmy[:, blk * 64 : (blk + 1) * 64],
                    scalar=rcpC,
                    in1=tsub[:, blk * 64 : (blk + 1) * 64],
                    op0=ALU.mult,
                    op1=ALU.mult,
                    accum_out=pos_sb[:, blk : blk + 1],
                )

            # ---------------- main loop ----------------
            rd_cols = cp.tile([128, nblk * njt], FP)
            lab_ap = lab_d.ap()
            for blk in range(nblk):
                lhsT_blk = lhsT16[:, blk * 128 : (blk + 1) * 128]
                for d in range(nld):
                    L = lp_pool.tile([128, ldw], FP, tag="L")
                    nc.sync.dma_start(
                        out=L[:],
                        in_=lab_ap[
                            blk * 128 : (blk + 1) * 128, d * ldw : (d + 1) * ldw
                        ],
                    )
                    for h in range(kl_per_ld):
                        jt = d * kl_per_ld + h
                        kl = kl_ps.tile([128, 1024], FP, tag="kl")
                        for half in range(2):
                            j0 = jt * 1024 + half * 512
                            nc.tensor.matmul(
                                kl[:, half * 512 : (half + 1) * 512],
                                lhsT_blk,
                                rhs[:, j0 : j0 + 512],
                                start=True,
                                stop=True,
                            )
                        scr = sp.tile([128, 1024], FP, tag="scr")
                        nc.vector.scalar_tensor_tensor(
                            out=scr[:],
                            in0=kl[:],
                            scalar=1.0,
                            in1=L[:, h * 1024 : (h + 1) * 1024],
                            op0=ALU.mult,
                            op1=ALU.mult,
                            accum_out=rd_cols[:, blk * njt + jt : blk * njt + jt + 1],
                        )

            # ---------------- epilogue ----------------
            rd8 = cp.tile([128, nblk], FP)
            nc.vector.reduce_sum(
                rd8[:],
                rd_cols[:].rearrange("p (b j) -> p b j", j=njt),
                axis=AX.X,
            )
            neg8 = cp.tile([128, nblk], FP)
            nc.vector.scalar_tensor_tensor(
                out=neg8[:],
                in0=rskl_sb[:],
                scalar=2.0,
                in1=rd8[:],
                op0=ALU.mult,
                op1=ALU.subtract,
            )
            rec8 = cp.tile([128, nblk], FP)
            nc.vector.reciprocal(rec8[:], neg8[:])
            r8 = cp.tile([128, nblk], FP)
            nc.vector.tensor_tensor(r8[:], pos_sb[:], rec8[:], op=ALU.mult)
            out_col = cp.tile([128, 1], FP)
            nc.vector.reduce_sum(out_col[:], r8[:], axis=AX.X)
            nc.sync.dma_start(out=out_d.ap(), in_=out_col[:])

    nc.compile()
    return nc


_NC_CACHE = {}


def _get_nc(B, shard):
    key = (B, shard)
    if key not in _NC_CACHE:
        _NC_CACHE[key] = build_nc(B, shard)
    return _NC_CACHE[key]


def make_in_maps(q, p, labels_matrix, n_cores=N_CORES):
    B = q.shape[0]
    shard = B // n_cores
    maps = []
    for k in range(n_cores):
        s = slice(k * shard, (k + 1) * shard)
        maps.append(
            {
                "q": np.ascontiguousarray(q[s]),
                "p": np.ascontiguousarray(p),
                "p_my": np.ascontiguousarray(p[s]),
                "labels": np.ascontiguousarray(labels_matrix[s]),
            }
        )
    return maps


def kernel(q, p, labels_matrix):
    from concourse.bass_utils import run_bass_kernel_spmd

    q = np.asarray(q, dtype=np.float32)
    p = np.asarray(p, dtype=np.float32)
    labels_matrix = np.asarray(labels_matrix, dtype=np.float32)
    B = q.shape[0]
    shard = B // N_CORES
    nc = _get_nc(B, shard)
    in_maps = make_in_maps(q, p, labels_matrix, N_CORES)
    res = run_bass_kernel_spmd(nc, in_maps, core_ids=list(range(N_CORES)))
    total = 0.0
    for r in res.results:
        total += r["out"].astype(np.float64).sum()
    return np.float32(total)


# revision 18
# speedup vs baseline: 2.1441x; 1.7212x over previous
"""Trainium2 Bass kernel for nn_DistributionLossWithLabel.

Reference computation (B=8192, C=64):
    lq = log(q); lp = log(p)
    positive[i] = mean_c p[i,c]*(lp[i,c]-lq[i,c])
    a[j]        = sum_c p[j,c]*lp[j,c] / C
    kl[i,j]     = a[j] - (lq @ p^T)[i,j] / C
    negative[i] = sum_j kl[i,j] + sum_j kl[i,j]*(1-L[i,j])
    loss        = sum_i positive[i]/negative[i]

Device reformulation (rows i sharded 8 ways, D = 2 - L shipped from host
transposed as bf16; {1,2} and {0,1} are exact in bf16):
    negative[i] = sum_j kl[i,j]*(2-L[i,j])
                = (D@a)[i] - sum_c (lq[i,c]/C) * (D@p)[i,c]
    [Dp | Da] accumulates on the TensorEngine as paug^T @ D^T where
    paug = [p | a_hi | a_lo] (bf16, with a carried as a hi/lo split to
    kill the bf16 rounding of the dominant term), streamed against D^T
    tiles straight from HBM.  The 8192x8192 KL matrix never exists, the
    VectorEngine only does O(B) epilogue work, and the kernel is bound by
    reading D^T once (16MB/core).
"""

import sys

if "/opt/trn_rl_repo" not in sys.path:
    sys.path.insert(0, "/opt/trn_rl_repo")

import ml_dtypes
import numpy as np

import concourse.bass as bass
import concourse.tile as tile
from concourse import bacc, mybir
from concourse.masks import make_identity

FP = mybir.dt.float32
BF = mybir.dt.bfloat16
AF = mybir.ActivationFunctionType
ALU = mybir.AluOpType
AX = mybir.AxisListType

B_FULL = 8192
C = 64
N_CORES = 8
NAUG = 66  # 64 p columns + a_hi + a_lo


def build_nc(B=B_FULL, shard=B_FULL // N_CORES, debug=False):
    """Build the single-core SPMD Bass program.

    B: total rows (j extent), multiple of 512.
    shard: rows per core (i extent), multiple of 128.
    """
    assert B % 512 == 0 and shard % 128 == 0
    njc = B // 128           # 128-row j-chunks of p / D^T
    nblk = shard // 128      # 128-row i-blocks
    nhalf = (shard + 511) // 512
    ccpt = 4                 # j-chunks per D^T DMA tile
    assert njc % ccpt == 0
    rcpC = 1.0 / C

    nc = bacc.Bacc("TRN2", target_bir_lowering=False, debug=debug)

    q_d = nc.dram_tensor("q", [shard, C], FP, kind="ExternalInput")
    p_d = nc.dram_tensor("p", [B, C], FP, kind="ExternalInput")
    pmy_d = nc.dram_tensor("p_my", [shard, C], FP, kind="ExternalInput")
    # D^T = (2 - labels)^T for this core's row shard: [B, shard] bf16
    lab_d = nc.dram_tensor("labels", [B, shard], BF, kind="ExternalInput")
    out_d = nc.dram_tensor("out", [128, 1], FP, kind="ExternalOutput")

    with tile.TileContext(nc) as tc:
        with (
            tc.tile_pool(name="const", bufs=1) as cp,
            tc.tile_pool(name="lpool", bufs=6) as lp_pool,
            tc.tile_pool(name="spool", bufs=2) as sp,
            tc.tile_pool(name="mps_ps", bufs=1, space="PSUM") as mps_ps,
            tc.tile_pool(name="tr_ps", bufs=2, space="PSUM") as tr_ps,
        ):
            ident = cp.tile([128, 128], FP)
            make_identity(nc, ident[:])

            # ---------------- p prologue -> paug ----------------
            P_nat = cp.tile([128, njc * 64], FP)
            nc.sync.dma_start(
                out=P_nat[:].rearrange("p (n c) -> p n c", c=64),
                in_=p_d.ap().rearrange("(n p) c -> p n c", p=128),
            )
            LP = cp.tile([128, njc * 64], FP)
            nc.scalar.activation(LP[:], P_nat[:], AF.Ln)
            A = cp.tile([128, njc * 64], FP)
            nc.vector.tensor_tensor(A[:], P_nat[:], LP[:], op=ALU.mult)
            asum = cp.tile([128, njc], FP)  # sum_c p*lp (unscaled)
            nc.vector.reduce_sum(
                asum[:], A[:].rearrange("p (n c) -> p n c", c=64), axis=AX.X
            )

            paug = cp.tile([128, njc * NAUG], BF)
            paug_v = paug[:].rearrange("p (n w) -> p n w", w=NAUG)
            nc.scalar.copy(
                paug_v[:, :, 0:64], P_nat[:].rearrange("p (n c) -> p n c", c=64)
            )
            # a_hi = bf16(a), a_lo = bf16(a - a_hi); a = asum/C
            nc.scalar.activation(
                paug_v[:, :, 64:65],
                asum[:].rearrange("p (n o) -> p n o", o=1),
                AF.Copy,
                scale=rcpC,
            )
            ah32 = cp.tile([128, njc], FP)
            nc.vector.tensor_copy(
                ah32[:].rearrange("p (n o) -> p n o", o=1), paug_v[:, :, 64:65]
            )
            alo = cp.tile([128, njc], FP)
            nc.vector.scalar_tensor_tensor(
                out=alo[:],
                in0=asum[:],
                scalar=rcpC,
                in1=ah32[:],
                op0=ALU.mult,
                op1=ALU.subtract,
            )
            nc.scalar.copy(paug_v[:, :, 65:66], alo[:].rearrange("p (n o) -> p n o", o=1))

            # ---------------- q / positive prologue ----------------
            QRAW = cp.tile([128, nblk * 64], FP)
            nc.sync.dma_start(
                out=QRAW[:].rearrange("p (n c) -> p n c", c=64),
                in_=q_d.ap().rearrange("(n p) c -> p n c", p=128),
            )
            lq = cp.tile([128, nblk * 64], FP)
            nc.scalar.activation(lq[:], QRAW[:], AF.Ln)

            Pmy = cp.tile([128, nblk * 64], FP)
            nc.sync.dma_start(
                out=Pmy[:].rearrange("p (n c) -> p n c", c=64),
                in_=pmy_d.ap().rearrange("(n p) c -> p n c", p=128),
            )
            LPmy = cp.tile([128, nblk * 64], FP)
            nc.scalar.activation(LPmy[:], Pmy[:], AF.Ln)
            tsub = cp.tile([128, nblk * 64], FP)
            nc.vector.tensor_tensor(tsub[:], LPmy[:], lq[:], op=ALU.subtract)
            pos_sb = cp.tile([128, nblk], FP)
            for blk in range(nblk):
                pscr = sp.tile([128, 64], FP, tag="pscr")
                nc.vector.scalar_tensor_tensor(
                    out=pscr[:],
                    in0=Pmy[:, blk * 64 : (blk + 1) * 64],
                    scalar=rcpC,
                    in1=tsub[:, blk * 64 : (blk + 1) * 64],
                    op0=ALU.mult,
                    op1=ALU.mult,
                    accum_out=pos_sb[:, blk : blk + 1],
                )

            # ---------------- main loop: [Dp|Da]^T += paug^T @ D^T ----------
            mps = mps_ps.tile([128, shard], FP)
            lab_ap = lab_d.ap()
            for g in range(njc // ccpt):
                Lt = lp_pool.tile([128, ccpt, shard], BF, tag="L")
                nc.sync.dma_start(
                    out=Lt[:],
                    in_=lab_ap[
                        g * ccpt * 128 : (g + 1) * ccpt * 128, :
                    ].rearrange("(cc p) i -> p cc i", p=128),
                )
                for cc in range(ccpt):
                    ch = g * ccpt + cc
                    lw = paug[:, ch * NAUG : (ch + 1) * NAUG]
                    for h in range(nhalf):
                        i0 = h * 512
                        iw = min(512, shard - i0)
                        nc.tensor.matmul(
                            mps[0:NAUG, i0 : i0 + iw],
                            lw,
                            Lt[:, cc, i0 : i0 + iw],
                            start=(ch == 0),
                            stop=(ch == njc - 1),
                        )

            # ---------------- epilogue ----------------
            DpT = cp.tile([128, shard], FP)
            nc.scalar.copy(DpT[0:NAUG, :], mps[0:NAUG, :])
            updp = cp.tile([128, nblk], FP)
            da2 = cp.tile([128, nblk * 2], FP)
            for blk in range(nblk):
                tr = tr_ps.tile([128, NAUG], FP, tag="tr")
                nc.tensor.transpose(
                    tr[:],
                    DpT[0:NAUG, blk * 128 : (blk + 1) * 128],
                    ident[0:NAUG, 0:NAUG],
                )
                escr = sp.tile([128, 64], FP, tag="escr")
                nc.vector.scalar_tensor_tensor(
                    out=escr[:],
                    in0=tr[:, 0:64],
                    scalar=rcpC,
                    in1=lq[:, blk * 64 : (blk + 1) * 64],
                    op0=ALU.mult,
                    op1=ALU.mult,
                    accum_out=updp[:, blk : blk + 1],
                )
                nc.scalar.copy(da2[:, blk * 2 : (blk + 1) * 2], tr[:, 64:66])
            da_sb = cp.tile([128, nblk], FP)
            da2v = da2[:].rearrange("p (n t) -> p n t", t=2)
            nc.vector.tensor_tensor(
                da_sb[:].rearrange("p (n o) -> p n o", o=1),
                da2v[:, :, 0:1],
                da2v[:, :, 1:2],
                op=ALU.add,
            )
            neg8 = cp.tile([128, nblk], FP)
            nc.vector.scalar_tensor_tensor(
                out=neg8[:],
                in0=updp[:],
                scalar=-1.0,
                in1=da_sb[:],
                op0=ALU.mult,
                op1=ALU.add,
            )
            rec8 = cp.tile([128, nblk], FP)
            nc.vector.reciprocal(rec8[:], neg8[:])
            r8 = cp.tile([128, nblk], FP)
            nc.vector.tensor_tensor(r8[:], pos_sb[:], rec8[:], op=ALU.mult)
            out_col = cp.tile([128, 1], FP)
            nc.vector.reduce_sum(out_col[:], r8[:], axis=AX.X)
            nc.sync.dma_start(out=out_d.ap(), in_=out_col[:])

    nc.compile()
    return nc


_NC_CACHE = {}


def _get_nc(B, shard):
    key = (B, shard)
    if key not in _NC_CACHE:
        _NC_CACHE[key] = build_nc(B, shard)
    return _NC_CACHE[key]


def make_dt(labels_shard):
    """(2 - labels)^T as contiguous bf16 [B, shard]."""
    return (2.0 - labels_shard).T.astype(ml_dtypes.bfloat16, order="C")


def make_in_maps(q, p, labels_matrix, n_cores=N_CORES):
    B = q.shape[0]
    shard = B // n_cores
    maps = []
    for k in range(n_cores):
        s = slice(k * shard, (k + 1) * shard)
        maps.append(
            {
                "q": np.ascontiguousarray(q[s]),
                "p": np.ascontiguousarray(p),
                "p_my": np.ascontiguousarray(p[s]),
                "labels": make_dt(labels_matrix[s]),
            }
        )
    return maps


def kernel(q, p, labels_matrix):
    from concourse.bass_utils import run_bass_kernel_spmd

    q = np.asarray(q, dtype=np.float32)
    p = np.asarray(p, dtype=np.float32)
    labels_matrix = np.asarray(labels_matrix, dtype=np.float32)
    B = q.shape[0]
    shard = B // N_CORES
    nc = _get_nc(B, shard)
    in_maps = make_in_maps(q, p, labels_matrix, N_CORES)
    res = run_bass_kernel_spmd(nc, in_maps, core_ids=list(range(N_CORES)))
    total = 0.0
    for r in res.results:
        total += r["out"].astype(np.float64).sum()
    return np.float32(total)


# revision 21
# speedup vs baseline: 2.1911x; 1.0219x over previous
"""Trainium2 Bass kernel for nn_DistributionLossWithLabel.

Reference computation (B=8192, C=64):
    lq = log(q); lp = log(p)
    positive[i] = mean_c p[i,c]*(lp[i,c]-lq[i,c])
    a[j]        = sum_c p[j,c]*lp[j,c] / C
    kl[i,j]     = a[j] - (lq @ p^T)[i,j] / C
    negative[i] = sum_j kl[i,j] + sum_j kl[i,j]*(1-L[i,j])
    loss        = sum_i positive[i]/negative[i]

Device reformulation (rows i sharded 8 ways, D = 2 - L shipped from host
transposed as bf16; {1,2} and {0,1} are exact in bf16):
    negative[i] = sum_j kl[i,j]*(2-L[i,j])
                = (D@a)[i] - sum_c (lq[i,c]/C) * (D@p)[i,c]
    [Dp | Da] accumulates on the TensorEngine as paug^T @ D^T where
    paug = [p | a_hi | a_lo] (bf16, with a carried as a hi/lo split to
    kill the bf16 rounding of the dominant term), streamed against D^T
    tiles straight from HBM.  The 8192x8192 KL matrix never exists, the
    VectorEngine only does O(B) epilogue work, and the kernel is bound by
    reading D^T once (16MB/core).
"""

import sys

if "/opt/trn_rl_repo" not in sys.path:
    sys.path.insert(0, "/opt/trn_rl_repo")

import ml_dtypes
import numpy as np

import concourse.bass as bass
import concourse.tile as tile
from concourse import bacc, mybir
from concourse.masks import make_identity

FP = mybir.dt.float32
BF = mybir.dt.bfloat16
AF = mybir.ActivationFunctionType
ALU = mybir.AluOpType
AX = mybir.AxisListType

B_FULL = 8192
C = 64
N_CORES = 8
NAUG = 66  # 64 p columns + a_hi + a_lo


def build_nc(B=B_FULL, shard=B_FULL // N_CORES, debug=False):
    """Build the single-core SPMD Bass program.

    B: total rows (j extent), multiple of 512.
    shard: rows per core (i extent), multiple of 128.
    """
    assert B % 512 == 0 and shard % 128 == 0
    njc = B // 128           # 128-row j-chunks of p / D^T
    nblk = shard // 128      # 128-row i-blocks
    nhalf = (shard + 511) // 512
    ccpt = 4                 # j-chunks per D^T DMA tile
    assert njc % ccpt == 0
    rcpC = 1.0 / C

    nc = bacc.Bacc("TRN2", target_bir_lowering=False, debug=debug)

    q_d = nc.dram_tensor("q", [shard, C], FP, kind="ExternalInput")
    p_d = nc.dram_tensor("p", [B, C], FP, kind="ExternalInput")
    pmy_d = nc.dram_tensor("p_my", [shard, C], FP, kind="ExternalInput")
    # D^T = (2 - labels)^T for this core's row shard: [B, shard] bf16
    lab_d = nc.dram_tensor("labels", [B, shard], BF, kind="ExternalInput")
    out_d = nc.dram_tensor("out", [128, 1], FP, kind="ExternalOutput")

    with tile.TileContext(nc) as tc:
        with (
            tc.tile_pool(name="const", bufs=1) as cp,
            tc.tile_pool(name="lpool", bufs=8) as lp_pool,
            tc.tile_pool(name="spool", bufs=2) as sp,
            tc.tile_pool(name="mps_ps", bufs=1, space="PSUM") as mps_ps,
            tc.tile_pool(name="tr_ps", bufs=2, space="PSUM") as tr_ps,
        ):
            ident = cp.tile([128, 128], FP)
            make_identity(nc, ident[:])

            # ---------------- p prologue -> paug (pipelined quarters) -------
            # Quarter-granular ops + subtile deps let main-loop matmuls on
            # early chunks start while later quarters are still loading.
            P_nat = cp.tile([128, njc * 64], FP)
            LP = cp.tile([128, njc * 64], FP)
            A = cp.tile([128, njc * 64], FP)
            asum = cp.tile([128, njc], FP)  # sum_c p*lp (unscaled)
            ah32 = cp.tile([128, njc], FP)
            alo = cp.tile([128, njc], FP)
            paug = cp.tile([128, njc * NAUG], BF)
            paug_v = paug[:].rearrange("p (n w) -> p n w", w=NAUG)

            NQ = 4
            qw = njc // NQ
            p_r = p_d.ap().rearrange("(n p) c -> p n c", p=128)
            for qd in range(NQ):
                ns = slice(qd * qw, (qd + 1) * qw)
                fs = slice(qd * qw * 64, (qd + 1) * qw * 64)
                eng = nc.sync if qd % 2 == 0 else nc.gpsimd
                eng.dma_start(
                    out=P_nat[:, fs].rearrange("p (n c) -> p n c", c=64),
                    in_=p_r[:, ns, :],
                )
                nc.scalar.activation(LP[:, fs], P_nat[:, fs], AF.Ln)
                nc.vector.tensor_tensor(
                    A[:, fs], P_nat[:, fs], LP[:, fs], op=ALU.mult
                )
                nc.vector.reduce_sum(
                    asum[:, ns],
                    A[:, fs].rearrange("p (n c) -> p n c", c=64),
                    axis=AX.X,
                )
                nc.scalar.copy(
                    paug_v[:, ns, 0:64],
                    P_nat[:, fs].rearrange("p (n c) -> p n c", c=64),
                )
                # a_hi = bf16(a), a_lo = bf16(a - a_hi); a = asum/C
                nc.scalar.activation(
                    paug_v[:, ns, 64:65],
                    asum[:, ns].rearrange("p (n o) -> p n o", o=1),
                    AF.Copy,
                    scale=rcpC,
                )
                nc.vector.tensor_copy(
                    ah32[:, ns].rearrange("p (n o) -> p n o", o=1),
                    paug_v[:, ns, 64:65],
                )
                nc.vector.scalar_tensor_tensor(
                    out=alo[:, ns],
                    in0=asum[:, ns],
                    scalar=rcpC,
                    in1=ah32[:, ns],
                    op0=ALU.mult,
                    op1=ALU.subtract,
                )
                nc.scalar.copy(
                    paug_v[:, ns, 65:66],
                    alo[:, ns].rearrange("p (n o) -> p n o", o=1),
                )

            # ---------------- main loop: [Dp|Da]^T += paug^T @ D^T ----------
            mps = mps_ps.tile([128, shard], FP)
            lab_ap = lab_d.ap()
            for g in range(njc // ccpt):
                Lt = lp_pool.tile([128, ccpt, shard], BF, tag="L")
                eng = nc.sync if g % 2 == 0 else nc.gpsimd
                eng.dma_start(
                    out=Lt[:],
                    in_=lab_ap[
                        g * ccpt * 128 : (g + 1) * ccpt * 128, :
                    ].rearrange("(cc p) i -> p cc i", p=128),
                )
                for cc in range(ccpt):
                    ch = g * ccpt + cc
                    lw = paug[:, ch * NAUG : (ch + 1) * NAUG]
                    for h in range(nhalf):
                        i0 = h * 512
                        iw = min(512, shard - i0)
                        nc.tensor.matmul(
                            mps[0:NAUG, i0 : i0 + iw],
                            lw,
                            Lt[:, cc, i0 : i0 + iw],
                            start=(ch == 0),
                            stop=(ch == njc - 1),
                        )

            # ---------------- q / positive (overlaps main loop) ------------
            QRAW = cp.tile([128, nblk * 64], FP)
            nc.sync.dma_start(
                out=QRAW[:].rearrange("p (n c) -> p n c", c=64),
                in_=q_d.ap().rearrange("(n p) c -> p n c", p=128),
            )
            lq = cp.tile([128, nblk * 64], FP)
            nc.scalar.activation(lq[:], QRAW[:], AF.Ln)

            Pmy = cp.tile([128, nblk * 64], FP)
            nc.gpsimd.dma_start(
                out=Pmy[:].rearrange("p (n c) -> p n c", c=64),
                in_=pmy_d.ap().rearrange("(n p) c -> p n c", p=128),
            )
            LPmy = cp.tile([128, nblk * 64], FP)
            nc.scalar.activation(LPmy[:], Pmy[:], AF.Ln)
            tsub = cp.tile([128, nblk * 64], FP)
            nc.vector.tensor_tensor(tsub[:], LPmy[:], lq[:], op=ALU.subtract)
            pos_sb = cp.tile([128, nblk], FP)
            for blk in range(nblk):
                pscr = sp.tile([128, 64], FP, tag="pscr")
                nc.vector.scalar_tensor_tensor(
                    out=pscr[:],
                    in0=Pmy[:, blk * 64 : (blk + 1) * 64],
                    scalar=rcpC,
                    in1=tsub[:, blk * 64 : (blk + 1) * 64],
                    op0=ALU.mult,
                    op1=ALU.mult,
                    accum_out=pos_sb[:, blk : blk + 1],
                )

            # ---------------- epilogue ----------------
            DpT = cp.tile([128, shard], FP)
            nc.scalar.copy(DpT[0:NAUG, :], mps[0:NAUG, :])
            updp = cp.tile([128, nblk], FP)
            da2 = cp.tile([128, nblk * 2], FP)
            for blk in range(nblk):
                tr = tr_ps.tile([128, NAUG], FP, tag="tr")
                nc.tensor.transpose(
                    tr[:],
                    DpT[0:NAUG, blk * 128 : (blk + 1) * 128],
                    ident[0:NAUG, 0:NAUG],
                )
                escr = sp.tile([128, 64], FP, tag="escr")
                nc.vector.scalar_tensor_tensor(
                    out=escr[:],
                    in0=tr[:, 0:64],
                    scalar=rcpC,
                    in1=lq[:, blk * 64 : (blk + 1) * 64],
                    op0=ALU.mult,
                    op1=ALU.mult,
                    accum_out=updp[:, blk : blk + 1],
                )
                nc.scalar.copy(da2[:, blk * 2 : (blk + 1) * 2], tr[:, 64:66])
            da_sb = cp.tile([128, nblk], FP)
            da2v = da2[:].rearrange("p (n t) -> p n t", t=2)
            nc.vector.tensor_tensor(
                da_sb[:].rearrange("p (n o) -> p n o", o=1),
                da2v[:, :, 0:1],
                da2v[:, :, 1:2],
                op=ALU.add,
            )
            neg8 = cp.tile([128, nblk], FP)
            nc.vector.scalar_tensor_tensor(
                out=neg8[:],
                in0=updp[:],
                scalar=-1.0,
                in1=da_sb[:],
                op0=ALU.mult,
                op1=ALU.add,
            )
            rec8 = cp.tile([128, nblk], FP)
            nc.vector.reciprocal(rec8[:], neg8[:])
            r8 = cp.tile([128, nblk], FP)
            nc.vector.tensor_tensor(r8[:], pos_sb[:], rec8[:], op=ALU.mult)
            out_col = cp.tile([128, 1], FP)
            nc.vector.reduce_sum(out_col[:], r8[:], axis=AX.X)
            nc.sync.dma_start(out=out_d.ap(), in_=out_col[:])

    nc.compile()
    return nc


_NC_CACHE = {}


def _get_nc(B, shard):
    key = (B, shard)
    if key not in _NC_CACHE:
        _NC_CACHE[key] = build_nc(B, shard)
    return _NC_CACHE[key]


def make_dt(labels_shard):
    """(2 - labels)^T as contiguous bf16 [B, shard]."""
    return (2.0 - labels_shard).T.astype(ml_dtypes.bfloat16, order="C")


def make_in_maps(q, p, labels_matrix, n_cores=N_CORES):
    B = q.shape[0]
    shard = B // n_cores
    maps = []
    for k in range(n_cores):
        s = slice(k * shard, (k + 1) * shard)
        maps.append(
            {
                "q": np.ascontiguousarray(q[s]),
                "p": np.ascontiguousarray(p),
                "p_my": np.ascontiguousarray(p[s]),
                "labels": make_dt(labels_matrix[s]),
            }
        )
    return maps


def kernel(q, p, labels_matrix):
    from concourse.bass_utils import run_bass_kernel_spmd

    q = np.asarray(q, dtype=np.float32)
    p = np.asarray(p, dtype=np.float32)
    labels_matrix = np.asarray(labels_matrix, dtype=np.float32)
    B = q.shape[0]
    shard = B // N_CORES
    nc = _get_nc(B, shard)
    in_maps = make_in_maps(q, p, labels_matrix, N_CORES)
    res = run_bass_kernel_spmd(nc, in_maps, core_ids=list(range(N_CORES)))
    total = 0.0
    for r in res.results:
        total += r["out"].astype(np.float64).sum()
    return np.float32(total)


# revision 25
# speedup vs baseline: 2.2153x; 1.0110x over previous
"""Trainium2 Bass kernel for nn_DistributionLossWithLabel.

Reference computation (B=8192, C=64):
    lq = log(q); lp = log(p)
    positive[i] = mean_c p[i,c]*(lp[i,c]-lq[i,c])
    a[j]        = sum_c p[j,c]*lp[j,c] / C
    kl[i,j]     = a[j] - (lq @ p^T)[i,j] / C
    negative[i] = sum_j kl[i,j] + sum_j kl[i,j]*(1-L[i,j])
    loss        = sum_i positive[i]/negative[i]

Device reformulation (rows i sharded 8 ways, D = 2 - L shipped from host
transposed as bf16; {1,2} and {0,1} are exact in bf16):
    negative[i] = sum_j kl[i,j]*(2-L[i,j])
                = (D@a)[i] - sum_c (lq[i,c]/C) * (D@p)[i,c]
    [Dp | Da] accumulates on the TensorEngine as paug^T @ D^T where
    paug = [p | a_hi | a_lo] (bf16, with a carried as a hi/lo split to
    kill the bf16 rounding of the dominant term), streamed against D^T
    tiles straight from HBM.  The 8192x8192 KL matrix never exists, the
    VectorEngine only does O(B) epilogue work, and the kernel is bound by
    reading D^T once (16MB/core).
"""

import sys

if "/opt/trn_rl_repo" not in sys.path:
    sys.path.insert(0, "/opt/trn_rl_repo")

import ml_dtypes
import numpy as np

import concourse.bass as bass
import concourse.tile as tile
from concourse import bacc, mybir
from concourse.masks import make_identity

FP = mybir.dt.float32
BF = mybir.dt.bfloat16
AF = mybir.ActivationFunctionType
ALU = mybir.AluOpType
AX = mybir.AxisListType

B_FULL = 8192
C = 64
N_CORES = 8
NAUG = 66  # 64 p columns + a_hi + a_lo


def build_nc(B=B_FULL, shard=B_FULL // N_CORES, debug=False):
    """Build the single-core SPMD Bass program.

    B: total rows (j extent), multiple of 512.
    shard: rows per core (i extent), multiple of 128.
    """
    assert B % 512 == 0 and shard % 128 == 0
    njc = B // 128           # 128-row j-chunks of p / D^T
    nblk = shard // 128      # 128-row i-blocks
    nhalf = (shard + 511) // 512
    ccpt = 4                 # j-chunks per D^T DMA tile
    assert njc % ccpt == 0
    rcpC = 1.0 / C

    nc = bacc.Bacc("TRN2", target_bir_lowering=False, debug=debug)

    # q/p/p_my arrive pre-chunked from host: [128, nchunks*64] where
    # partition pp, col n*64+c = row n*128+pp, col c — so every input DMA
    # is contiguous per partition (line rate) and rows land on partitions.
    q_d = nc.dram_tensor("q", [128, nblk * 64], FP, kind="ExternalInput")
    p_d = nc.dram_tensor("p", [128, njc * 64], FP, kind="ExternalInput")
    pmy_d = nc.dram_tensor("p_my", [128, nblk * 64], FP, kind="ExternalInput")
    # D^T = (2 - labels)^T for this core's row shard: [B, shard] bf16
    lab_d = nc.dram_tensor("labels", [B, shard], BF, kind="ExternalInput")
    out_d = nc.dram_tensor("out", [128, 1], FP, kind="ExternalOutput")

    with tile.TileContext(nc) as tc:
        with (
            tc.tile_pool(name="const", bufs=1) as cp,
            tc.tile_pool(name="lpool", bufs=8) as lp_pool,
            tc.tile_pool(name="spool", bufs=2) as sp,
            tc.tile_pool(name="mps_ps", bufs=1, space="PSUM") as mps_ps,
            tc.tile_pool(name="tr_ps", bufs=2, space="PSUM") as tr_ps,
        ):
            ident = cp.tile([128, 128], FP)
            make_identity(nc, ident[:])

            # ---------------- p prologue -> paug (pipelined quarters) -------
            # Quarter-granular ops + subtile deps let main-loop matmuls on
            # early chunks start while later quarters are still loading.
            P_nat = cp.tile([128, njc * 64], FP)
            LP = cp.tile([128, njc * 64], FP)
            A = cp.tile([128, njc * 64], FP)
            asum = cp.tile([128, njc], FP)  # sum_c p*lp (unscaled)
            ah32 = cp.tile([128, njc], FP)
            alo = cp.tile([128, njc], FP)
            paug = cp.tile([128, njc * NAUG], BF)
            paug_v = paug[:].rearrange("p (n w) -> p n w", w=NAUG)

            NQ = 4
            qw = njc // NQ
            p_ap = p_d.ap()
            for qd in range(NQ):
                ns = slice(qd * qw, (qd + 1) * qw)
                fs = slice(qd * qw * 64, (qd + 1) * qw * 64)
                nc.gpsimd.dma_start(out=P_nat[:, fs], in_=p_ap[:, fs])
                nc.scalar.activation(LP[:, fs], P_nat[:, fs], AF.Ln)
                nc.vector.tensor_tensor(
                    A[:, fs], P_nat[:, fs], LP[:, fs], op=ALU.mult
                )
                nc.vector.reduce_sum(
                    asum[:, ns],
                    A[:, fs].rearrange("p (n c) -> p n c", c=64),
                    axis=AX.X,
                )
                nc.scalar.copy(
                    paug_v[:, ns, 0:64],
                    P_nat[:, fs].rearrange("p (n c) -> p n c", c=64),
                )
                # a_hi = bf16(a), a_lo = bf16(a - a_hi); a = asum/C
                nc.scalar.activation(
                    paug_v[:, ns, 64:65],
                    asum[:, ns].rearrange("p (n o) -> p n o", o=1),
                    AF.Copy,
                    scale=rcpC,
                )
                nc.vector.tensor_copy(
                    ah32[:, ns].rearrange("p (n o) -> p n o", o=1),
                    paug_v[:, ns, 64:65],
                )
                nc.vector.scalar_tensor_tensor(
                    out=alo[:, ns],
                    in0=asum[:, ns],
                    scalar=rcpC,
                    in1=ah32[:, ns],
                    op0=ALU.mult,
                    op1=ALU.subtract,
                )
                nc.scalar.copy(
                    paug_v[:, ns, 65:66],
                    alo[:, ns].rearrange("p (n o) -> p n o", o=1),
                )

            # ---------------- main loop: [Dp|Da]^T += paug^T @ D^T ----------
            mps = mps_ps.tile([128, shard], FP)
            lab_ap = lab_d.ap()
            for g in range(njc // ccpt):
                Lt = lp_pool.tile([128, ccpt, shard], BF, tag="L")
                eng = nc.sync if g % 2 == 0 else nc.gpsimd
                eng.dma_start(
                    out=Lt[:],
                    in_=lab_ap[
                        g * ccpt * 128 : (g + 1) * ccpt * 128, :
                    ].rearrange("(cc p) i -> p cc i", p=128),
                )
                for cc in range(ccpt):
                    ch = g * ccpt + cc
                    lw = paug[:, ch * NAUG : (ch + 1) * NAUG]
                    for h in range(nhalf):
                        i0 = h * 512
                        iw = min(512, shard - i0)
                        nc.tensor.matmul(
                            mps[0:NAUG, i0 : i0 + iw],
                            lw,
                            Lt[:, cc, i0 : i0 + iw],
                            start=(ch == 0),
                            stop=(ch == njc - 1),
                        )

            # ---------------- q / positive (overlaps main loop) ------------
            QRAW = cp.tile([128, nblk * 64], FP)
            nc.gpsimd.dma_start(out=QRAW[:], in_=q_d.ap())
            lq = cp.tile([128, nblk * 64], FP)
            nc.scalar.activation(lq[:], QRAW[:], AF.Ln)

            Pmy = cp.tile([128, nblk * 64], FP)
            nc.gpsimd.dma_start(out=Pmy[:], in_=pmy_d.ap())
            LPmy = cp.tile([128, nblk * 64], FP)
            nc.scalar.activation(LPmy[:], Pmy[:], AF.Ln)
            tsub = cp.tile([128, nblk * 64], FP)
            nc.vector.tensor_tensor(tsub[:], LPmy[:], lq[:], op=ALU.subtract)
            pos_sb = cp.tile([128, nblk], FP)
            for blk in range(nblk):
                pscr = sp.tile([128, 64], FP, tag="pscr")
                nc.vector.scalar_tensor_tensor(
                    out=pscr[:],
                    in0=Pmy[:, blk * 64 : (blk + 1) * 64],
                    scalar=rcpC,
                    in1=tsub[:, blk * 64 : (blk + 1) * 64],
                    op0=ALU.mult,
                    op1=ALU.mult,
                    accum_out=pos_sb[:, blk : blk + 1],
                )

            # ---------------- epilogue ----------------
            DpT = cp.tile([128, shard], FP)
            nc.scalar.copy(DpT[0:NAUG, :], mps[0:NAUG, :])
            updp = cp.tile([128, nblk], FP)
            da2 = cp.tile([128, nblk * 2], FP)
            for blk in range(nblk):
                tr = tr_ps.tile([128, NAUG], FP, tag="tr")
                nc.tensor.transpose(
                    tr[:],
                    DpT[0:NAUG, blk * 128 : (blk + 1) * 128],
                    ident[0:NAUG, 0:NAUG],
                )
                escr = sp.tile([128, 64], FP, tag="escr")
                nc.vector.scalar_tensor_tensor(
                    out=escr[:],
                    in0=tr[:, 0:64],
                    scalar=rcpC,
                    in1=lq[:, blk * 64 : (blk + 1) * 64],
                    op0=ALU.mult,
                    op1=ALU.mult,
                    accum_out=updp[:, blk : blk + 1],
                )
                nc.scalar.copy(da2[:, blk * 2 : (blk + 1) * 2], tr[:, 64:66])
            da_sb = cp.tile([128, nblk], FP)
            da2v = da2[:].rearrange("p (n t) -> p n t", t=2)
            nc.vector.tensor_tensor(
                da_sb[:].rearrange("p (n o) -> p n o", o=1),
                da2v[:, :, 0:1],
                da2v[:, :, 1:2],
                op=ALU.add,
            )
            neg8 = cp.tile([128, nblk], FP)
            nc.vector.scalar_tensor_tensor(
                out=neg8[:],
                in0=updp[:],
                scalar=-1.0,
                in1=da_sb[:],
                op0=ALU.mult,
                op1=ALU.add,
            )
            rec8 = cp.tile([128, nblk], FP)
            nc.vector.reciprocal(rec8[:], neg8[:])
            r8 = cp.tile([128, nblk], FP)
            nc.vector.tensor_tensor(r8[:], pos_sb[:], rec8[:], op=ALU.mult)
            out_col = cp.tile([128, 1], FP)
            nc.vector.reduce_sum(out_col[:], r8[:], axis=AX.X)
            nc.sync.dma_start(out=out_d.ap(), in_=out_col[:])

    nc.compile()
    return nc


_NC_CACHE = {}


def _get_nc(B, shard):
    key = (B, shard)
    if key not in _NC_CACHE:
        _NC_CACHE[key] = build_nc(B, shard)
    return _NC_CACHE[key]


def make_dt(labels_shard):
    """(2 - labels)^T as contiguous bf16 [B, shard]."""
    return (2.0 - labels_shard).T.astype(ml_dtypes.bfloat16, order="C")


def chunk_rows(arr):
    """[N, 64] fp32 -> [128, (N/128)*64]: partition pp, col n*64+c = row
    n*128+pp — the on-chip chunked layout, pre-computed on host so the
    DMA is a contiguous line-rate load."""
    n = arr.shape[0] // 128
    return np.ascontiguousarray(
        arr.reshape(n, 128, 64).transpose(1, 0, 2).reshape(128, n * 64)
    )


def make_in_maps(q, p, labels_matrix, n_cores=N_CORES):
    B = q.shape[0]
    shard = B // n_cores
    maps = []
    p_ch = chunk_rows(p)
    for k in range(n_cores):
        s = slice(k * shard, (k + 1) * shard)
        maps.append(
            {
                "q": chunk_rows(q[s]),
                "p": p_ch,
                "p_my": chunk_rows(p[s]),
                "labels": make_dt(labels_matrix[s]),
            }
        )
    return maps


def kernel(q, p, labels_matrix):
    from concourse.bass_utils import run_bass_kernel_spmd

    q = np.asarray(q, dtype=np.float32)
    p = np.asarray(p, dtype=np.float32)
    labels_matrix = np.asarray(labels_matrix, dtype=np.float32)
    B = q.shape[0]
    shard = B // N_CORES
    nc = _get_nc(B, shard)
    in_maps = make_in_maps(q, p, labels_matrix, N_CORES)
    res = run_bass_kernel_spmd(nc, in_maps, core_ids=list(range(N_CORES)))
    total = 0.0
    for r in res.results:
        total += r["out"].astype(np.float64).sum()
    return np.float32(total)


# revision 26
# speedup vs baseline: 2.4068x; 1.0865x over previous
"""Trainium2 Bass kernel for nn_DistributionLossWithLabel.

Reference computation (B=8192, C=64):
    lq = log(q); lp = log(p)
    positive[i] = mean_c p[i,c]*(lp[i,c]-lq[i,c])
    a[j]        = sum_c p[j,c]*lp[j,c] / C
    kl[i,j]     = a[j] - (lq @ p^T)[i,j] / C
    negative[i] = sum_j kl[i,j] + sum_j kl[i,j]*(1-L[i,j])
    loss        = sum_i positive[i]/negative[i]

Device reformulation (rows i sharded 8 ways, D = 2 - L shipped from host
transposed as bf16; {1,2} and {0,1} are exact in bf16):
    negative[i] = sum_j kl[i,j]*(2-L[i,j])
                = (D@a)[i] - sum_c (lq[i,c]/C) * (D@p)[i,c]
    [Dp | Da] accumulates on the TensorEngine as paug^T @ D^T where
    paug = [p | a_hi | a_lo] (bf16, with a carried as a hi/lo split to
    kill the bf16 rounding of the dominant term), streamed against D^T
    tiles straight from HBM.  The 8192x8192 KL matrix never exists, the
    VectorEngine only does O(B) epilogue work, and the kernel is bound by
    reading D^T once (16MB/core).
"""

import sys

if "/opt/trn_rl_repo" not in sys.path:
    sys.path.insert(0, "/opt/trn_rl_repo")

import ml_dtypes
import numpy as np

import concourse.bass as bass
import concourse.tile as tile
from concourse import bacc, mybir
from concourse.masks import make_identity

FP = mybir.dt.float32
BF = mybir.dt.bfloat16
F8 = mybir.dt.float8e4
AF = mybir.ActivationFunctionType
ALU = mybir.AluOpType
AX = mybir.AxisListType

B_FULL = 8192
C = 64
N_CORES = 8
NAUG = 66  # 64 p columns + a_hi + a_lo


def build_nc(B=B_FULL, shard=B_FULL // N_CORES, debug=False):
    """Build the single-core SPMD Bass program.

    B: total rows (j extent), multiple of 512.
    shard: rows per core (i extent), multiple of 128.
    """
    assert B % 512 == 0 and shard % 128 == 0
    njc = B // 128           # 128-row j-chunks of p / D^T
    nblk = shard // 128      # 128-row i-blocks
    nhalf = (shard + 511) // 512
    ccpt = 4                 # j-chunks per D^T DMA tile
    assert njc % ccpt == 0
    rcpC = 1.0 / C

    nc = bacc.Bacc("TRN2", target_bir_lowering=False, debug=debug)

    # q/p/p_my arrive pre-chunked from host: [128, nchunks*64] where
    # partition pp, col n*64+c = row n*128+pp, col c — so every input DMA
    # is contiguous per partition (line rate) and rows land on partitions.
    q_d = nc.dram_tensor("q", [128, nblk * 64], FP, kind="ExternalInput")
    p_d = nc.dram_tensor("p", [128, njc * 64], FP, kind="ExternalInput")
    pmy_d = nc.dram_tensor("p_my", [128, nblk * 64], FP, kind="ExternalInput")
    # D^T = (2 - labels)^T for this core's row shard: [B, shard] fp8e4m3
    # ({1,2} are exact in e4m3; the PE takes bf16 weights x fp8 moving)
    lab_d = nc.dram_tensor("labels", [B, shard], F8, kind="ExternalInput")
    out_d = nc.dram_tensor("out", [128, 1], FP, kind="ExternalOutput")

    with tile.TileContext(nc) as tc:
        with (
            tc.tile_pool(name="const", bufs=1) as cp,
            tc.tile_pool(name="lpool", bufs=8) as lp_pool,
            tc.tile_pool(name="spool", bufs=2) as sp,
            tc.tile_pool(name="mps_ps", bufs=1, space="PSUM") as mps_ps,
            tc.tile_pool(name="tr_ps", bufs=2, space="PSUM") as tr_ps,
        ):
            ident = cp.tile([128, 128], FP)
            make_identity(nc, ident[:])

            # ---------------- p prologue -> paug (pipelined quarters) -------
            # Quarter-granular ops + subtile deps let main-loop matmuls on
            # early chunks start while later quarters are still loading.
            P_nat = cp.tile([128, njc * 64], FP)
            LP = cp.tile([128, njc * 64], FP)
            A = cp.tile([128, njc * 64], FP)
            asum = cp.tile([128, njc], FP)  # sum_c p*lp (unscaled)
            ah32 = cp.tile([128, njc], FP)
            alo = cp.tile([128, njc], FP)
            paug = cp.tile([128, njc * NAUG], BF)
            paug_v = paug[:].rearrange("p (n w) -> p n w", w=NAUG)

            NQ = 8
            qw = njc // NQ
            p_ap = p_d.ap()
            for qd in range(NQ):
                ns = slice(qd * qw, (qd + 1) * qw)
                fs = slice(qd * qw * 64, (qd + 1) * qw * 64)
                nc.sync.dma_start(out=P_nat[:, fs], in_=p_ap[:, fs])
                nc.scalar.activation(LP[:, fs], P_nat[:, fs], AF.Ln)
                nc.vector.tensor_tensor(
                    A[:, fs], P_nat[:, fs], LP[:, fs], op=ALU.mult
                )
                nc.vector.reduce_sum(
                    asum[:, ns],
                    A[:, fs].rearrange("p (n c) -> p n c", c=64),
                    axis=AX.X,
                )
                nc.scalar.copy(
                    paug_v[:, ns, 0:64],
                    P_nat[:, fs].rearrange("p (n c) -> p n c", c=64),
                )
                # a_hi = bf16(a), a_lo = bf16(a - a_hi); a = asum/C
                nc.scalar.activation(
                    paug_v[:, ns, 64:65],
                    asum[:, ns].rearrange("p (n o) -> p n o", o=1),
                    AF.Copy,
                    scale=rcpC,
                )
                nc.vector.tensor_copy(
                    ah32[:, ns].rearrange("p (n o) -> p n o", o=1),
                    paug_v[:, ns, 64:65],
                )
                nc.vector.scalar_tensor_tensor(
                    out=alo[:, ns],
                    in0=asum[:, ns],
                    scalar=rcpC,
                    in1=ah32[:, ns],
                    op0=ALU.mult,
                    op1=ALU.subtract,
                )
                nc.scalar.copy(
                    paug_v[:, ns, 65:66],
                    alo[:, ns].rearrange("p (n o) -> p n o", o=1),
                )

            # ---------------- main loop: [Dp|Da]^T += paug^T @ D^T ----------
            mps = mps_ps.tile([128, shard], FP)
            lab_ap = lab_d.ap()
            for g in range(njc // ccpt):
                Lt = lp_pool.tile([128, ccpt, shard], F8, tag="L")
                eng = nc.gpsimd if g % 2 == 0 else nc.sync
                eng.dma_start(
                    out=Lt[:],
                    in_=lab_ap[
                        g * ccpt * 128 : (g + 1) * ccpt * 128, :
                    ].rearrange("(cc p) i -> p cc i", p=128),
                )
                for cc in range(ccpt):
                    ch = g * ccpt + cc
                    lw = paug[:, ch * NAUG : (ch + 1) * NAUG]
                    for h in range(nhalf):
                        i0 = h * 512
                        iw = min(512, shard - i0)
                        nc.tensor.matmul(
                            mps[0:NAUG, i0 : i0 + iw],
                            lw,
                            Lt[:, cc, i0 : i0 + iw],
                            start=(ch == 0),
                            stop=(ch == njc - 1),
                        )

            # ---------------- q / positive (overlaps main loop) ------------
            QRAW = cp.tile([128, nblk * 64], FP)
            nc.gpsimd.dma_start(out=QRAW[:], in_=q_d.ap())
            lq = cp.tile([128, nblk * 64], FP)
            nc.scalar.activation(lq[:], QRAW[:], AF.Ln)

            Pmy = cp.tile([128, nblk * 64], FP)
            nc.gpsimd.dma_start(out=Pmy[:], in_=pmy_d.ap())
            LPmy = cp.tile([128, nblk * 64], FP)
            nc.scalar.activation(LPmy[:], Pmy[:], AF.Ln)
            tsub = cp.tile([128, nblk * 64], FP)
            nc.vector.tensor_tensor(tsub[:], LPmy[:], lq[:], op=ALU.subtract)
            pos_sb = cp.tile([128, nblk], FP)
            for blk in range(nblk):
                pscr = sp.tile([128, 64], FP, tag="pscr")
                nc.vector.scalar_tensor_tensor(
                    out=pscr[:],
                    in0=Pmy[:, blk * 64 : (blk + 1) * 64],
                    scalar=rcpC,
                    in1=tsub[:, blk * 64 : (blk + 1) * 64],
                    op0=ALU.mult,
                    op1=ALU.mult,
                    accum_out=pos_sb[:, blk : blk + 1],
                )

            # ---------------- epilogue ----------------
            DpT = cp.tile([128, shard], FP)
            nc.scalar.copy(DpT[0:NAUG, :], mps[0:NAUG, :])
            updp = cp.tile([128, nblk], FP)
            da2 = cp.tile([128, nblk * 2], FP)
            for blk in range(nblk):
                tr = tr_ps.tile([128, NAUG], FP, tag="tr")
                nc.tensor.transpose(
                    tr[:],
                    DpT[0:NAUG, blk * 128 : (blk + 1) * 128],
                    ident[0:NAUG, 0:NAUG],
                )
                escr = sp.tile([128, 64], FP, tag="escr")
                nc.vector.scalar_tensor_tensor(
                    out=escr[:],
                    in0=tr[:, 0:64],
                    scalar=rcpC,
                    in1=lq[:, blk * 64 : (blk + 1) * 64],
                    op0=ALU.mult,
                    op1=ALU.mult,
                    accum_out=updp[:, blk : blk + 1],
                )
                nc.scalar.copy(da2[:, blk * 2 : (blk + 1) * 2], tr[:, 64:66])
            da_sb = cp.tile([128, nblk], FP)
            da2v = da2[:].rearrange("p (n t) -> p n t", t=2)
            nc.vector.tensor_tensor(
                da_sb[:].rearrange("p (n o) -> p n o", o=1),
                da2v[:, :, 0:1],
                da2v[:, :, 1:2],
                op=ALU.add,
            )
            neg8 = cp.tile([128, nblk], FP)
            nc.vector.scalar_tensor_tensor(
                out=neg8[:],
                in0=updp[:],
                scalar=-1.0,
                in1=da_sb[:],
                op0=ALU.mult,
                op1=ALU.add,
            )
            rec8 = cp.tile([128, nblk], FP)
            nc.vector.reciprocal(rec8[:], neg8[:])
            r8 = cp.tile([128, nblk], FP)
            nc.vector.tensor_tensor(r8[:], pos_sb[:], rec8[:], op=ALU.mult)
            out_col = cp.tile([128, 1], FP)
            nc.vector.reduce_sum(out_col[:], r8[:], axis=AX.X)
            nc.sync.dma_start(out=out_d.ap(), in_=out_col[:])

    nc.compile()
    return nc


_NC_CACHE = {}


def _get_nc(B, shard):
    key = (B, shard)
    if key not in _NC_CACHE:
        _NC_CACHE[key] = build_nc(B, shard)
    return _NC_CACHE[key]


def make_dt(labels_shard):
    """(2 - labels)^T as contiguous fp8e4m3 [B, shard]."""
    return (2.0 - labels_shard).T.astype(ml_dtypes.float8_e4m3, order="C")


def chunk_rows(arr):
    """[N, 64] fp32 -> [128, (N/128)*64]: partition pp, col n*64+c = row
    n*128+pp — the on-chip chunked layout, pre-computed on host so the
    DMA is a contiguous line-rate load."""
    n = arr.shape[0] // 128
    return np.ascontiguousarray(
        arr.reshape(n, 128, 64).transpose(1, 0, 2).reshape(128, n * 64)
    )


def make_in_maps(q, p, labels_matrix, n_cores=N_CORES):
    B = q.shape[0]
    shard = B // n_cores
    maps = []
    p_ch = chunk_rows(p)
    for k in range(n_cores):
        s = slice(k * shard, (k + 1) * shard)
        maps.append(
            {
                "q": chunk_rows(q[s]),
                "p": p_ch,
                "p_my": chunk_rows(p[s]),
                "labels": make_dt(labels_matrix[s]),
            }
        )
    return maps


def kernel(q, p, labels_matrix):
    from concourse.bass_utils import run_bass_kernel_spmd

    q = np.asarray(q, dtype=np.float32)
    p = np.asarray(p, dtype=np.float32)
    labels_matrix = np.asarray(labels_matrix, dtype=np.float32)
    B = q.shape[0]
    shard = B // N_CORES
    nc = _get_nc(B, shard)
    in_maps = make_in_maps(q, p, labels_matrix, N_CORES)
    res = run_bass_kernel_spmd(nc, in_maps, core_ids=list(range(N_CORES)))
    total = 0.0
    for r in res.results:
        total += r["out"].astype(np.float64).sum()
    return np.float32(total)
